# revision 1
# baseline (speedup 1.0000x reference)
"""Ragged chunk-slice gather (chunked-prefill KV index gather) on 8 trn2 cores.

Problem: out[t] = req_to_token[req_pool_indices[seg(t)],
                               chunk_starts[seg(t)] + (t - cu[seg(t)])]
where seg(t) is the request owning flat token t (ragged by cu_seq_lens).

Sharding (data/request parallel per the hint): core k owns requests
[k*8, (k+1)*8). Its shard of the req_to_token pool table is the 8 rows those
requests reference (host-side row sharding, ~1 MB/core). On device each core
gathers its local requests' chunk slices: 8 dynamic-start DRAM->DRAM DMA
copies of the full MAX_CHUNK window (always in-bounds since
start <= MAX_CONTEXT - MAX_CHUNK). Host then slices each request's valid
prefix and concatenates by cu_seq_len offsets (the all-gather step).
"""

import numpy as np

import concourse.bass as bass
import concourse.mybir as mybir
from concourse.bass_utils import run_bass_kernel_spmd
from concourse.ordered_set import OrderedSet

N_CORES = 8
BATCH = 64
RPC = BATCH // N_CORES          # requests per core
MAX_CONTEXT = 32768             # req_to_token row length
MAX_CHUNK = 4096                # max tokens per request chunk
POOL_SIZE = 4096                # req_to_token rows
MAX_START = MAX_CONTEXT - MAX_CHUNK

_CACHE = {}
LAST_RESULTS = None             # BassKernelResults of the most recent run


def _build_nc():
    nc = bass.Bass("TRN2")
    rows = nc.dram_tensor(
        "rows", [RPC, MAX_CONTEXT], mybir.dt.int32, kind="ExternalInput")
    starts = nc.dram_tensor(
        "starts", [1, RPC], mybir.dt.int32, kind="ExternalInput")
    out = nc.dram_tensor(
        "out", [RPC, MAX_CHUNK], mybir.dt.int32, kind="ExternalOutput")

    with (
        nc.Block() as block,
        nc.semaphore("dma_sem") as dma_sem,
        nc.sbuf_tensor([1, RPC], mybir.dt.int32) as sb_starts,
    ):
        @block.sync
        def _(sync):
            sync.dma_start(sb_starts[:, :], starts[:, :]).then_inc(dma_sem, 16)
            sync.wait_ge(dma_sem, 16)
            _, vals = nc.values_load_multi_w_load_instructions(
                sb_starts[0:1, :],
                engines=OrderedSet([mybir.EngineType.SP]),
                min_val=0,
                max_val=MAX_START,
                skip_runtime_bounds_check=True,
            )
            for i in range(RPC):
                sync.dma_start(
                    out[i:i + 1, :],
                    rows[i:i + 1, bass.ds(vals[i], MAX_CHUNK)],
                ).then_inc(dma_sem, 16)
            sync.wait_ge(dma_sem, 16 * (RPC + 1))

    return nc


def _reference_fallback(r2t, rpi, starts, cu, T):
    """Exact (clamped-gather) mirror of the jax reference, for inputs that
    violate the setup_inputs invariants. Pure numpy."""
    t = np.arange(T, dtype=np.int64)
    seg = np.searchsorted(cu.astype(np.int64), t, side="right") - 1
    seg_c = np.clip(seg, 0, BATCH - 1)
    pos = t - cu.astype(np.int64)[np.clip(seg, -len(cu), len(cu) - 1)]
    rows = rpi.astype(np.int64)[seg_c]
    cols = starts.astype(np.int64)[seg_c] + pos
    rows = np.clip(rows, 0, r2t.shape[0] - 1)
    cols = np.clip(cols, 0, r2t.shape[1] - 1)
    return r2t[rows, cols].astype(np.int32)


def kernel(req_to_token, req_pool_indices, chunk_starts, chunk_seq_lens,
           chunk_cu_seq_lens, num_chunk_tokens):
    global LAST_RESULTS
    r2t = np.asarray(req_to_token, dtype=np.int32)
    rpi = np.asarray(req_pool_indices, dtype=np.int64)
    starts = np.asarray(chunk_starts, dtype=np.int64)
    cu = np.asarray(chunk_cu_seq_lens, dtype=np.int64)
    T = int(num_chunk_tokens)

    # Per-request valid lengths from cu offsets (truncated at T).
    lens = np.minimum(cu[1:], T) - cu[:-1]
    lens = np.clip(lens, 0, None)

    fast = (
        r2t.shape == (POOL_SIZE, MAX_CONTEXT)
        and rpi.shape == (BATCH,)
        and starts.shape == (BATCH,)
        and cu.shape == (BATCH + 1,)
        and cu[0] == 0
        and np.all(np.diff(cu) >= 0)
        and T <= int(cu[-1])
        and np.all(lens <= MAX_CHUNK)
        and np.all(rpi >= 0) and np.all(rpi < POOL_SIZE)
        and np.all(starts >= 0)
        and np.all(starts + lens <= MAX_CONTEXT)
    )
    if not fast:
        return _reference_fallback(r2t, rpi, starts, cu, T)

    # Window starts the device copies: [wstart, wstart + MAX_CHUNK) must be
    # in-bounds and contain [start, start + len).
    wstarts = np.minimum(starts, MAX_START)
    delta = (starts - wstarts).astype(np.int64)   # valid data offset in window

    if "nc" not in _CACHE:
        _CACHE["nc"] = _build_nc()
    nc = _CACHE["nc"]

    # Shard: core k gets requests [k*RPC, (k+1)*RPC) and the table rows they
    # reference.
    in_maps = []
    for k in range(N_CORES):
        sl = slice(k * RPC, (k + 1) * RPC)
        in_maps.append({
            "rows": np.ascontiguousarray(r2t[rpi[sl]]),
            "starts": wstarts[sl].astype(np.int32).reshape(1, RPC),
        })

    res = run_bass_kernel_spmd(nc, in_maps, core_ids=list(range(N_CORES)))
    LAST_RESULTS = res

    # All-gather the ragged outputs by cu_seq_len offsets.
    out = np.empty(T, dtype=np.int32)
    for k in range(N_CORES):
        core_out = res.results[k]["out"]
        for j in range(RPC):
            i = k * RPC + j
            li = int(lens[i])
            if li > 0:
                d = int(delta[i])
                out[cu[i]:cu[i] + li] = core_out[j, d:d + li]
    return out


# revision 2
# speedup vs baseline: 1.1493x; 1.1493x over previous
"""Ragged chunk-slice gather (chunked-prefill KV index gather) on 8 trn2 cores.

Problem: out[t] = req_to_token[req_pool_indices[seg(t)],
                               chunk_starts[seg(t)] + (t - cu[seg(t)])]
where seg(t) is the request owning flat token t (ragged by cu_seq_lens).

Sharding (data/request parallel per the hint): core k owns requests
[k*8, (k+1)*8). Its shard of the req_to_token pool table is the 8 rows those
requests reference (host-side row sharding, ~1 MB/core). On device each core
gathers its local requests' chunk slices: 8 dynamic-start DRAM->DRAM DMA
copies of the full MAX_CHUNK window (always in-bounds since
start <= MAX_CONTEXT - MAX_CHUNK). Host then slices each request's valid
prefix and concatenates by cu_seq_len offsets (the all-gather step).
"""

import numpy as np

import concourse.bass as bass
import concourse.mybir as mybir
from concourse.bass_utils import run_bass_kernel_spmd
from concourse.ordered_set import OrderedSet

N_CORES = 8
BATCH = 64
RPC = BATCH // N_CORES          # requests per core
MAX_CONTEXT = 32768             # req_to_token row length
MAX_CHUNK = 4096                # max tokens per request chunk
POOL_SIZE = 4096                # req_to_token rows
MAX_START = MAX_CONTEXT - MAX_CHUNK

_CACHE = {}
LAST_RESULTS = None             # BassKernelResults of the most recent run


def _build_nc():
    nc = bass.Bass("TRN2", enable_partition_id=False)
    rows = nc.dram_tensor(
        "rows", [RPC, MAX_CONTEXT], mybir.dt.int32, kind="ExternalInput")
    starts = nc.dram_tensor(
        "starts", [1, RPC], mybir.dt.int32, kind="ExternalInput")
    out = nc.dram_tensor(
        "out", [RPC, MAX_CHUNK], mybir.dt.int32, kind="ExternalOutput")

    HALF = RPC // 2
    with (
        nc.Block() as block,
        nc.semaphore("dma_sem") as dma_sem,
        nc.sbuf_tensor([1, RPC], mybir.dt.int32) as sb_starts,
    ):
        def issue_half(eng, eng_type, lo):
            # each HWDGE engine loads its own start registers, then issues
            # its half of the dynamic-offset gather DMAs
            eng.wait_ge(dma_sem, 16)
            _, vals = nc.values_load_multi_w_load_instructions(
                sb_starts[0:1, lo:lo + HALF],
                engines=OrderedSet([eng_type]),
                min_val=0,
                max_val=MAX_START,
                skip_runtime_bounds_check=True,
            )
            for k in range(HALF):
                i = lo + k
                eng.dma_start(
                    out[i:i + 1, :],
                    rows[i:i + 1, bass.ds(vals[k], MAX_CHUNK)],
                ).then_inc(dma_sem, 16)

        @block.scalar
        def _(scalar):
            scalar.dma_start(
                sb_starts[:, :], starts[:, :]).then_inc(dma_sem, 16)
            issue_half(scalar, mybir.EngineType.Activation, HALF)

        @block.sync
        def _(sync):
            issue_half(sync, mybir.EngineType.SP, 0)
            # one engine waits for all 9 DMA completions (starts + 8 gathers)
            sync.wait_ge(dma_sem, 16 * (RPC + 1))

    return nc


def _reference_fallback(r2t, rpi, starts, cu, T):
    """Exact (clamped-gather) mirror of the jax reference, for inputs that
    violate the setup_inputs invariants. Pure numpy."""
    t = np.arange(T, dtype=np.int64)
    seg = np.searchsorted(cu.astype(np.int64), t, side="right") - 1
    seg_c = np.clip(seg, 0, BATCH - 1)
    pos = t - cu.astype(np.int64)[np.clip(seg, -len(cu), len(cu) - 1)]
    rows = rpi.astype(np.int64)[seg_c]
    cols = starts.astype(np.int64)[seg_c] + pos
    rows = np.clip(rows, 0, r2t.shape[0] - 1)
    cols = np.clip(cols, 0, r2t.shape[1] - 1)
    return r2t[rows, cols].astype(np.int32)


def kernel(req_to_token, req_pool_indices, chunk_starts, chunk_seq_lens,
           chunk_cu_seq_lens, num_chunk_tokens):
    global LAST_RESULTS
    r2t = np.asarray(req_to_token, dtype=np.int32)
    rpi = np.asarray(req_pool_indices, dtype=np.int64)
    starts = np.asarray(chunk_starts, dtype=np.int64)
    cu = np.asarray(chunk_cu_seq_lens, dtype=np.int64)
    T = int(num_chunk_tokens)

    # Per-request valid lengths from cu offsets (truncated at T).
    lens = np.minimum(cu[1:], T) - cu[:-1]
    lens = np.clip(lens, 0, None)

    fast = (
        r2t.shape == (POOL_SIZE, MAX_CONTEXT)
        and rpi.shape == (BATCH,)
        and starts.shape == (BATCH,)
        and cu.shape == (BATCH + 1,)
        and cu[0] == 0
        and np.all(np.diff(cu) >= 0)
        and T <= int(cu[-1])
        and np.all(lens <= MAX_CHUNK)
        and np.all(rpi >= 0) and np.all(rpi < POOL_SIZE)
        and np.all(starts >= 0)
        and np.all(starts + lens <= MAX_CONTEXT)
    )
    if not fast:
        return _reference_fallback(r2t, rpi, starts, cu, T)

    # Window starts the device copies: [wstart, wstart + MAX_CHUNK) must be
    # in-bounds and contain [start, start + len).
    wstarts = np.minimum(starts, MAX_START)
    delta = (starts - wstarts).astype(np.int64)   # valid data offset in window

    if "nc" not in _CACHE:
        _CACHE["nc"] = _build_nc()
    nc = _CACHE["nc"]

    # Shard: core k gets requests [k*RPC, (k+1)*RPC) and the table rows they
    # reference.
    in_maps = []
    for k in range(N_CORES):
        sl = slice(k * RPC, (k + 1) * RPC)
        in_maps.append({
            "rows": np.ascontiguousarray(r2t[rpi[sl]]),
            "starts": wstarts[sl].astype(np.int32).reshape(1, RPC),
        })

    res = run_bass_kernel_spmd(nc, in_maps, core_ids=list(range(N_CORES)))
    LAST_RESULTS = res

    # All-gather the ragged outputs by cu_seq_len offsets.
    out = np.empty(T, dtype=np.int32)
    for k in range(N_CORES):
        core_out = res.results[k]["out"]
        for j in range(RPC):
            i = k * RPC + j
            li = int(lens[i])
            if li > 0:
                d = int(delta[i])
                out[cu[i]:cu[i] + li] = core_out[j, d:d + li]
    return out


# revision 4
# speedup vs baseline: 1.3161x; 1.1451x over previous
"""Ragged chunk-slice gather (chunked-prefill KV index gather) on 8 trn2 cores.

Problem: out[t] = req_to_token[req_pool_indices[seg(t)],
                               chunk_starts[seg(t)] + (t - cu[seg(t)])]
where seg(t) is the request owning flat token t (ragged by cu_seq_lens).

Sharding (data/request parallel per the hint): core k owns requests
[k*8, (k+1)*8). Its shard of the req_to_token pool table is the 8 rows those
requests reference (host-side row sharding, ~1 MB/core). On device each core
gathers its local requests' chunk slices: 8 dynamic-start DRAM->DRAM DMA
copies of the full MAX_CHUNK window (always in-bounds since
start <= MAX_CONTEXT - MAX_CHUNK). Host then slices each request's valid
prefix and concatenates by cu_seq_len offsets (the all-gather step).
"""

import numpy as np

import concourse.bass as bass
import concourse.mybir as mybir
from concourse.bass_utils import run_bass_kernel_spmd
from concourse.ordered_set import OrderedSet

N_CORES = 8
BATCH = 64
RPC = BATCH // N_CORES          # requests per core
MAX_CONTEXT = 32768             # req_to_token row length
MAX_CHUNK = 4096                # max tokens per request chunk
POOL_SIZE = 4096                # req_to_token rows
MAX_START = MAX_CONTEXT - MAX_CHUNK

_CACHE = {}
LAST_RESULTS = None             # BassKernelResults of the most recent run


ROW_BYTES = MAX_CONTEXT * 4
WIN_BYTES = MAX_CHUNK * 4


def _build_nc():
    # Byte-granularity view: host ships flat byte offsets
    # boffs[i] = i*ROW_BYTES + start_i*4, so each gather DMA is a single
    # register-offset slice of the flat shard with no address arithmetic.
    nc = bass.Bass("TRN2", enable_partition_id=False)
    rows = nc.dram_tensor(
        "rows", [RPC * ROW_BYTES], mybir.dt.uint8, kind="ExternalInput")
    boffs = nc.dram_tensor(
        "boffs", [1, RPC], mybir.dt.int32, kind="ExternalInput")
    out = nc.dram_tensor(
        "out", [RPC, WIN_BYTES], mybir.dt.uint8, kind="ExternalOutput")

    HALF = RPC // 2
    with (
        nc.Block() as block,
        nc.semaphore("dma_sem") as dma_sem,
    ):
        def issue_half(eng_type, lo):
            # load this engine's byte offsets straight from DRAM into
            # registers, then issue its half of the gather DMAs
            eng = nc.engines[eng_type]
            _, vals = nc.values_load_multi_w_load_instructions(
                boffs[0:1, lo:lo + HALF],
                engines=OrderedSet([eng_type]),
                min_val=0,
                max_val=(RPC - 1) * ROW_BYTES + (MAX_START * 4),
                skip_runtime_bounds_check=True,
            )
            for k in range(HALF):
                i = lo + k
                eng.dma_start(
                    out[i:i + 1, :],
                    rows[bass.ds(vals[k], WIN_BYTES)],
                ).then_inc(dma_sem, 16)

        @block.scalar
        def _(scalar):
            issue_half(mybir.EngineType.Activation, HALF)

        @block.sync
        def _(sync):
            issue_half(mybir.EngineType.SP, 0)
            sync.wait_ge(dma_sem, 16 * RPC)

    return nc


def _reference_fallback(r2t, rpi, starts, cu, T):
    """Exact (clamped-gather) mirror of the jax reference, for inputs that
    violate the setup_inputs invariants. Pure numpy."""
    t = np.arange(T, dtype=np.int64)
    seg = np.searchsorted(cu.astype(np.int64), t, side="right") - 1
    seg_c = np.clip(seg, 0, BATCH - 1)
    pos = t - cu.astype(np.int64)[np.clip(seg, -len(cu), len(cu) - 1)]
    rows = rpi.astype(np.int64)[seg_c]
    cols = starts.astype(np.int64)[seg_c] + pos
    rows = np.clip(rows, 0, r2t.shape[0] - 1)
    cols = np.clip(cols, 0, r2t.shape[1] - 1)
    return r2t[rows, cols].astype(np.int32)


def kernel(req_to_token, req_pool_indices, chunk_starts, chunk_seq_lens,
           chunk_cu_seq_lens, num_chunk_tokens):
    global LAST_RESULTS
    r2t = np.asarray(req_to_token, dtype=np.int32)
    rpi = np.asarray(req_pool_indices, dtype=np.int64)
    starts = np.asarray(chunk_starts, dtype=np.int64)
    cu = np.asarray(chunk_cu_seq_lens, dtype=np.int64)
    T = int(num_chunk_tokens)

    # Per-request valid lengths from cu offsets (truncated at T).
    lens = np.minimum(cu[1:], T) - cu[:-1]
    lens = np.clip(lens, 0, None)

    fast = (
        r2t.shape == (POOL_SIZE, MAX_CONTEXT)
        and rpi.shape == (BATCH,)
        and starts.shape == (BATCH,)
        and cu.shape == (BATCH + 1,)
        and cu[0] == 0
        and np.all(np.diff(cu) >= 0)
        and T <= int(cu[-1])
        and np.all(lens <= MAX_CHUNK)
        and np.all(rpi >= 0) and np.all(rpi < POOL_SIZE)
        and np.all(starts >= 0)
        and np.all(starts + lens <= MAX_CONTEXT)
    )
    if not fast:
        return _reference_fallback(r2t, rpi, starts, cu, T)

    # Window starts the device copies: [wstart, wstart + MAX_CHUNK) must be
    # in-bounds and contain [start, start + len).
    wstarts = np.minimum(starts, MAX_START)
    delta = (starts - wstarts).astype(np.int64)   # valid data offset in window

    if "nc" not in _CACHE:
        _CACHE["nc"] = _build_nc()
    nc = _CACHE["nc"]

    # Shard: core k gets requests [k*RPC, (k+1)*RPC) and the table rows they
    # reference. Offsets are flat byte offsets into the shard.
    base = np.arange(RPC, dtype=np.int64) * ROW_BYTES
    in_maps = []
    for k in range(N_CORES):
        sl = slice(k * RPC, (k + 1) * RPC)
        shard = np.ascontiguousarray(r2t[rpi[sl]])
        in_maps.append({
            "rows": shard.reshape(-1).view(np.uint8),
            "boffs": (base + wstarts[sl] * 4).astype(np.int32).reshape(1, RPC),
        })

    res = run_bass_kernel_spmd(nc, in_maps, core_ids=list(range(N_CORES)))
    LAST_RESULTS = res

    # All-gather the ragged outputs by cu_seq_len offsets.
    out = np.empty(T, dtype=np.int32)
    for k in range(N_CORES):
        core_out = res.results[k]["out"].view(np.int32)   # [RPC, MAX_CHUNK]
        for j in range(RPC):
            i = k * RPC + j
            li = int(lens[i])
            if li > 0:
                d = int(delta[i])
                out[cu[i]:cu[i] + li] = core_out[j, d:d + li]
    return out
